# revision 2
# baseline (speedup 1.0000x reference)
"""Multi-head causal attention (B=2, S=2048, D=1024, H=16) on 8 TRN2 NeuronCores.
135.3us TimelineSim (baseline 184.4us).

Sharding: core c handles batch b = c // 4 and local head group g = c % 4
(global heads 4g..4g+3).  Each core computes its heads' QKV projections,
causal attention, and a partial output projection; host sums the 4 partials
per batch and adds b_out.

v2 design (vs baseline):
  - QK projection: fp8e4 DoubleRow matmuls, 3-pass hi/lo error compensation
    (x8@W8 + x8lo@W8 + x8@W8lo) -> q,k accurate to ~0.4%; f32r scores.
  - V projection: same 3-pass fp8 DR -> vn in bf16 (k-major, +ones col).
  - Scores: f32r, two heads packed per 128-partition psum tile, causal
    trimming at 256 granularity (f32r needs moving dim >= 256).
  - exp on ACT -> bf16 E tiles; triangle masks on DVE (bf16 2x mode).
  - AV transposed: out[q(128), 65] = E_block[k,q].T @ Vn[k, 65]; moving dim
    is only 65 cols -> ~2x fewer PE cycles than value-major AV.  Ones column
    of Vn gives the softmax denominator per q ON THE PARTITION, so
    normalization is a per-partition reciprocal + tensor_scalar multiply
    (no cross-partition broadcast needed at all).
  - values transposed back to [d, q] with PE transpose matmuls (bf16),
    bf16 output projection.
"""

from contextlib import ExitStack

import numpy as np
import ml_dtypes

import concourse.bass as bass
import concourse.mybir as mybir
import concourse.tile as tile
from concourse import bass_utils

F32 = mybir.dt.float32
F32R = mybir.dt.float32r
BF16 = mybir.dt.bfloat16
FP8 = mybir.dt.float8e4
EXP = mybir.ActivationFunctionType.Exp
COPY = mybir.ActivationFunctionType.Copy
DR = mybir.MatmulPerfMode.DoubleRow

E4 = ml_dtypes.float8_e4m3
BF = ml_dtypes.bfloat16

B, S, D, H = 2, 2048, 1024, 16
HD = D // H          # 64
HL = 4               # heads per core
N_CORES = 8
SC = S // 512        # 4 q-chunks of 512
KT = S // 128        # 16 k-tiles of 128

_CACHE = {}


def _round_f32r(x: np.ndarray) -> np.ndarray:
    """Round f32 to fp32r (11-bit mantissa, RNE) on host."""
    u = np.ascontiguousarray(x, dtype=np.float32).view(np.uint32)
    frac = u & np.uint32(0x00000FFF)
    base = u & np.uint32(0xFFFFF000)
    bit = np.uint32(0x00000800)
    lsb = np.uint32(0x00001000)
    roundup = (frac > bit) | ((frac == bit) & ((u & lsb) != 0))
    return np.where(roundup, base + lsb, base).view(np.float32)


_NO_HOIST = {
    "AllEngineBarrier",
    "EventSemaphore",
    "UnconditionalBranch",
    "CompareAndBranch",
    "BranchHint",
    "IndirectBranch",
    "Halt",
    "Call",
    "OverlayCall",
    "NoOp",
}


def _fix_sync_waits(nc):
    """walrus codegen holds only one sync-wait per engine instruction; hoist
    excess waits onto same-engine NoOps inserted right before."""
    for fn in nc.m.functions:
        for blk in fn.blocks:
            insts = blk.instructions
            out = []
            changed = False
            for inst in insts:
                si = inst.sync_info
                if si is not None and inst.opcode not in _NO_HOIST:
                    waits = list(si.on_wait)
                    if len(waits) > 1:
                        for j, w in enumerate(waits[:-1]):
                            nop = mybir.InstNoOp(name=f"{inst.name}-wfix{j}")
                            nop.engine = inst.engine
                            nop.sync_info = mybir.SyncInfo(on_wait=[w], on_update=[])
                            out.append(nop)
                        inst.sync_info = mybir.SyncInfo(
                            on_wait=[waits[-1]], on_update=list(si.on_update)
                        )
                        changed = True
                out.append(inst)
            if changed:
                blk.instructions = out


def _build(fix_waits=True, dbg=False):
    nc = bass.Bass("TRN2", target_bir_lowering=False, debug=False,
                   num_devices=N_CORES)
    if dbg:
        d_qT = nc.dram_tensor("d_qT", [128, 2, S], F32R, kind="ExternalOutput").ap()
        d_kT = nc.dram_tensor("d_kT", [128, 2, S], F32R, kind="ExternalOutput").ap()
        d_vn = nc.dram_tensor("d_vn", [128, KT, 4, 65], BF16,
                              kind="ExternalOutput").ap()
        d_e = nc.dram_tensor("d_e", [128, 2, 512], BF16, kind="ExternalOutput").ap()
        d_vst = nc.dram_tensor("d_vst", [128, 4, 4, 64], BF16,
                               kind="ExternalOutput").ap()
        d_vnT = nc.dram_tensor("d_vnT", [128, 2, S], BF16,
                               kind="ExternalOutput").ap()

    xq8 = nc.dram_tensor("xq8", [128, 4, 2, S], FP8, kind="ExternalInput").ap()
    xq8l = nc.dram_tensor("xq8l", [128, 4, 2, S], FP8, kind="ExternalInput").ap()
    xs8 = nc.dram_tensor("xs8", [128, 4, 2, S], FP8, kind="ExternalInput").ap()
    zro = nc.dram_tensor("zro", [128, 384], F32R, kind="ExternalInput").ap()
    wq8 = nc.dram_tensor("wq8", [128, 4, 2, 4, 128], FP8, kind="ExternalInput").ap()
    wq8l = nc.dram_tensor("wq8l", [128, 4, 2, 4, 128], FP8, kind="ExternalInput").ap()
    wq8s = nc.dram_tensor("wq8s", [128, 4, 2, 4, 128], FP8, kind="ExternalInput").ap()
    wv8 = nc.dram_tensor("wv8", [128, 4, 2, 256], FP8, kind="ExternalInput").ap()
    wv8l = nc.dram_tensor("wv8l", [128, 4, 2, 256], FP8, kind="ExternalInput").ap()
    wv8s = nc.dram_tensor("wv8s", [128, 4, 2, 256], FP8, kind="ExternalInput").ap()
    woutb = nc.dram_tensor("woutb", [128, 2, D], BF16, kind="ExternalInput").ap()
    bq = nc.dram_tensor("bq", [128, 4], F32, kind="ExternalInput").ap()
    bv = nc.dram_tensor("bv", [128, 4, 64], F32, kind="ExternalInput").ap()
    vone = nc.dram_tensor("vone", [128, KT, 4, 1], BF16, kind="ExternalInput").ap()
    cmask = nc.dram_tensor("cmask", [128, 128], BF16, kind="ExternalInput").ap()
    identb = nc.dram_tensor("identb", [128, 128], BF16, kind="ExternalInput").ap()
    outT = nc.dram_tensor("outT", [128, 8, S], BF16, kind="ExternalOutput").ap()

    with tile.TileContext(nc) as tc, ExitStack() as ctx:
        persist = ctx.enter_context(tc.tile_pool(name="persist", bufs=1))
        xpool = ctx.enter_context(tc.tile_pool(name="xp", bufs=3))
        epool = ctx.enter_context(tc.tile_pool(name="ep", bufs=8))
        spool = ctx.enter_context(tc.tile_pool(name="stp", bufs=3))
        opool = ctx.enter_context(tc.tile_pool(name="op", bufs=6))
        # psum (8 banks): sp 2x2-bank, po 1x2-bank, small (pq/pv/pu/tr) 2x1
        ps = ctx.enter_context(tc.tile_pool(name="ps", bufs=2, space="PSUM"))

        wq_sb = persist.tile([128, 4, 2, 4, 128], FP8, tag="wq")
        wql_sb = persist.tile([128, 4, 2, 4, 128], FP8, tag="wql")
        wqs_sb = persist.tile([128, 4, 2, 4, 128], FP8, tag="wqs")
        wv_sb = persist.tile([128, 4, 2, 256], FP8, tag="wv")
        wvl_sb = persist.tile([128, 4, 2, 256], FP8, tag="wvl")
        wvs_sb = persist.tile([128, 4, 2, 256], FP8, tag="wvs")
        zro_sb = persist.tile([128, 384], F32R, tag="zro")
        wo_sb = persist.tile([128, 2, D], BF16, tag="wo")
        bq_sb = persist.tile([128, 4], F32, tag="bq")
        bv_sb = persist.tile([128, 4, 64], F32, tag="bv")
        cm_sb = persist.tile([128, 128], BF16, tag="cm")
        id_sb = persist.tile([128, 128], BF16, tag="id")
        qT = persist.tile([128, 2, S], F32R, tag="qT")
        kT = persist.tile([128, 2, S], F32R, tag="kT")
        vn = persist.tile([128, KT, 4, 65], BF16, tag="vn")
        vnT = persist.tile([128, 2, S], BF16, tag="vnT")

        # first x chunk first so the first matmuls start early
        xc0 = xpool.tile([128, 4, 2, 512], FP8, tag="xc", name="xc0")
        xl0 = xpool.tile([128, 4, 2, 512], FP8, tag="xl", name="xl0")
        xs0 = xpool.tile([128, 4, 2, 512], FP8, tag="xs", name="xs0")
        nc.sync.dma_start(xc0[:], xq8[:, :, :, 0:512])
        nc.scalar.dma_start(bq_sb[:], bq)
        nc.scalar.dma_start(wq_sb[:], wq8)
        nc.sync.dma_start(xs0[:], xs8[:, :, :, 0:512])
        nc.scalar.dma_start(wql_sb[:], wq8l)
        nc.sync.dma_start(xl0[:], xq8l[:, :, :, 0:512])
        nc.scalar.dma_start(wqs_sb[:], wq8s)
        nc.scalar.dma_start(bv_sb[:], bv)
        nc.scalar.dma_start(wv_sb[:], wv8)
        nc.scalar.dma_start(wvl_sb[:], wv8l)
        nc.scalar.dma_start(wvs_sb[:], wv8s)
        nc.scalar.dma_start(cm_sb[:], cmask)
        nc.scalar.dma_start(zro_sb[:], zro)
        nc.scalar.dma_start(id_sb[:], identb)
        nc.scalar.dma_start(wo_sb[:], woutb)
        # ones column of vn (softmax denominators) via memset, not DMA
        # (a strided single-element-column DMA costs ~3.6us of DMA engines)
        nc.vector.memset(vn[:, :, :, 64:65], 1.0)
        xtiles = {0: (xc0, xl0, xs0)}

        def qkv_dma(qc):
            qs = slice(qc * 512, (qc + 1) * 512)
            xc = xpool.tile([128, 4, 2, 512], FP8, tag="xc", name=f"xc{qc}")
            xl = xpool.tile([128, 4, 2, 512], FP8, tag="xl", name=f"xl{qc}")
            xs = xpool.tile([128, 4, 2, 512], FP8, tag="xs", name=f"xs{qc}")
            nc.sync.dma_start(xc[:], xq8[:, :, :, qs])
            nc.sync.dma_start(xl[:], xq8l[:, :, :, qs])
            nc.sync.dma_start(xs[:], xs8[:, :, :, qs])
            xtiles[qc] = (xc, xl, xs)

        def qk_tile(qc, mt):
            qs = slice(qc * 512, (qc + 1) * 512)
            xc, xl, xs = xtiles[qc]
            pq = ps.tile([128, 512], F32, tag="q1", name=f"pq{qc}{mt}")
            passes = [(wq_sb, xc), (wql_sb, xs), (wqs_sb, xl)]
            i = 0
            for wsb, xsb in passes:
                for kp in range(4):
                    nc.tensor.matmul(
                        pq[:], wsb[:, kp, :, mt, :], xsb[:, kp, :, :],
                        start=(i == 0), stop=(i == 11), perf_mode=DR)
                    i += 1
            dst = (qT if mt < 2 else kT)[:, mt % 2, qs]
            nc.vector.tensor_scalar_add(dst, pq[:], bq_sb[:, mt:mt + 1])

        def v_tile(qc, j):
            st = 4 * qc + j
            xc, xl, xs = xtiles[qc]
            pv = ps.tile([128, 512], F32, tag="q1", name=f"pv{qc}{j}")
            passes = [(wv_sb, xc), (wvl_sb, xs), (wvs_sb, xl)]
            i = 0
            for wsb, xsb in passes:
                for kp in range(4):
                    nc.tensor.matmul(
                        pv[0:128, 0:256], xsb[:, kp, :, j * 128:(j + 1) * 128],
                        wsb[:, kp, :, :],
                        start=(i == 0), stop=(i == 11), perf_mode=DR)
                    i += 1
            nc.vector.tensor_add(
                vn[:, st, :, 0:64],
                pv[0:128, 0:256].rearrange("p (h d) -> p h d", h=4),
                bv_sb[:])

        def qkv_pieces(qc):
            ps_ = [lambda qc=qc: qkv_dma(qc)] if qc > 0 else []
            for mt in (0, 2):
                ps_.append(lambda qc=qc, mt=mt: qk_tile(qc, mt))
            for j in range(4):
                ps_.append(lambda qc=qc, j=j: v_tile(qc, j))
            for mt in (1, 3):
                ps_.append(lambda qc=qc, mt=mt: qk_tile(qc, mt))
            return ps_

        vst_tiles = {}

        def tr_piece(qc, qt):
            vst = vst_tiles[qc]
            for dh in range(2):
                ptr = ps.tile([128, 128], BF16, tag="q1", name=f"tr{qc}{qt}{dh}")
                nc.tensor.matmul(ptr[:], vst[:, qt, 2 * dh:2 * dh + 2, :],
                                 id_sb[:], is_transpose=True)
                nc.vector.tensor_copy(
                    vnT[:, dh, qc * 512 + qt * 128:qc * 512 + (qt + 1) * 128],
                    ptr[:])

        def op_piece(qc, m):
            qs = slice(qc * 512, (qc + 1) * 512)
            pu = ps.tile([128, 512], F32, tag="q1", name=f"pu{qc}{m}")
            for t in range(2):
                nc.tensor.matmul(pu[:], wo_sb[:, t, m * 128:(m + 1) * 128],
                                 vnT[:, t, qs], start=(t == 0), stop=(t == 1))
            ou = opool.tile([128, 512], BF16, tag="ou", name=f"ou{qc}{m}")
            if qc == SC - 1 and m % 2 == 1:
                nc.scalar.copy(ou[:], pu[:])   # tail: ACT is idle
            else:
                nc.vector.tensor_copy(ou[:], pu[:])
            nc.sync.dma_start(outT[:, m, qs], ou[:])

        def post_pieces(qc):
            ps_ = [lambda qc=qc, qt=qt: tr_piece(qc, qt) for qt in range(4)]
            ps_ += [lambda qc=qc, m=m: op_piece(qc, m) for m in range(8)]
            return ps_

        def attn_qc(qc, queue):
            """Emit attention for chunk qc, interleaving `queue` pieces (PE
            work for the next chunk's projections and the previous chunk's
            transposes/output projection) between ki steps so every engine
            stays fed while the exp (ACT) chain runs."""
            vst = spool.tile([128, 4, 4, 64], BF16, tag="vst", name=f"vs{qc}")
            vst_tiles[qc] = vst
            n_ki = 4 * qc + 4
            n_steps = 2 * n_ki + 2
            qi = 0
            emitted = 0.0

            def drain(frac):
                nonlocal qi, emitted
                emitted += frac
                while qi < len(queue) and qi < emitted:
                    queue[qi]()
                    qi += 1

            per_step = len(queue) / n_steps
            for hp in range(2):
                po = ps.tile([128, 2, 512], F32, tag="po", name=f"po{qc}{hp}", bufs=1)
                for i in range(2):
                    # one start=True matmul zeroes all four qt accumulation
                    # regions of this bank (psum pending-zero is bank-wide)
                    nc.tensor.matmul(po[:, i, 0:260], zro_sb[0:1, 0:128],
                                     zro_sb[0:1, 0:260], start=True, stop=False,
                                     skip_group_check=True)
                for ki in range(n_ki):
                    j = ki - 4 * qc  # >= 0 on diagonal tiles
                    o_exp = max(0, 128 * j)
                    o_sc = min(o_exp, 256)  # f32r moving dim must be >= 256
                    sp = ps.tile([128, 2, 512], F32, tag="s",
                                 name=f"sp{qc}{hp}{ki}")
                    for i in range(2):
                        vp = 64 * i
                        nc.tensor.matmul(
                            sp[:, i, o_sc:512],
                            kT[vp:vp + 64, hp, ki * 128:(ki + 1) * 128],
                            qT[vp:vp + 64, hp, qc * 512 + o_sc:(qc + 1) * 512],
                            start=True, stop=True, tile_position=(vp, 0))
                    e = epool.tile([128, 2, 512], BF16, tag="e",
                                   name=f"e{qc}{hp}{ki}")
                    nc.scalar.activation(e[:, :, o_exp:512], sp[:, :, o_exp:512],
                                         EXP, scale=0.125)
                    if j >= 0:  # diagonal: mask the [128,128] triangle block
                        for i in range(2):
                            es = e[:, i, o_exp:o_exp + 128]
                            nc.vector.tensor_mul(es, es, cm_sb[:])
                    # AV transposed: po[q, 65] += E_block.T @ Vn
                    for i in range(2):
                        for qt in range(max(0, j), 4):
                            nc.tensor.matmul(
                                po[:, i, qt * 65:qt * 65 + 65],
                                e[:, i, qt * 128:(qt + 1) * 128],
                                vn[:, ki, 2 * hp + i, :],
                                start=False, stop=(ki == 4 * qc + qt),
                                skip_group_check=True)
                    drain(per_step)
                # normalize: per-partition recip of denominators, then scale
                rc = spool.tile([128, 2, 4], F32, tag="rc", name=f"rc{qc}{hp}")
                for i in range(2):
                    dn = po[:, i, 0:260].rearrange("p (qt c) -> p qt c", c=65)
                    with nc.allow_low_precision(reason="softmax recip"):
                        nc.vector.reciprocal(rc[:, i, :], dn[:, 0:4, 64:65])
                    for qt in range(4):
                        nc.vector.tensor_scalar_mul(
                            vst[:, qt, 2 * hp + i, :],
                            po[:, i, qt * 65:qt * 65 + 64],
                            rc[:, i, qt:qt + 1])
                drain(1.0)
            drain(len(queue))

        for piece in qkv_pieces(0):
            piece()
        for c in range(SC):
            queue = []
            if c + 1 < SC:
                queue += qkv_pieces(c + 1)
            if c >= 1:
                queue += post_pieces(c - 1)
            attn_qc(c, queue)
        for piece in post_pieces(SC - 1):
            piece()
        if dbg:
            nc.sync.dma_start(d_vst, vst_tiles[0][:])
            nc.sync.dma_start(d_qT, qT[:])
            nc.sync.dma_start(d_kT, kT[:])
            nc.sync.dma_start(d_vn, vn[:])
            nc.sync.dma_start(d_vnT, vnT[:])

    if fix_waits:
        _fix_sync_waits(nc)
    return nc


def _get_nc():
    if "nc" not in _CACHE:
        _CACHE["nc"] = _build()
    return _CACHE["nc"]


def _dr_layout(xb):
    """[S, 1024] -> [128, 4, 2, S]: p=partition, kp=k-tile-pair, sl=slot."""
    return np.ascontiguousarray(
        xb.T.reshape(4, 2, 128, xb.shape[0]).transpose(2, 0, 1, 3))


def kernel(x, W_qkv, b_qkv, W_out, b_out):
    x = np.asarray(x, np.float32)
    W_qkv = np.asarray(W_qkv, np.float32)
    b_qkv = np.asarray(b_qkv, np.float32)
    W_out = np.asarray(W_out, np.float32)
    b_out = np.asarray(b_out, np.float32)

    nc = _get_nc()

    kk = np.arange(128)[:, None]
    qq = np.arange(128)[None, :]
    cmask = (kk <= qq).astype(BF)
    identb = np.eye(128, dtype=np.float32).astype(BF)
    vone = np.ones((128, KT, 4, 1), np.float32).astype(BF)

    in_maps = []
    for c in range(N_CORES):
        b, g = divmod(c, 4)
        heads = [4 * g + i for i in range(HL)]

        xb = x[b]                                        # [S, 1024]
        xr = _dr_layout(xb)
        x8 = xr.astype(E4)
        x8l = ((xr - x8.astype(np.float32)) * 8.0).astype(E4)
        xs8_a = (xr * 0.125).astype(E4)

        # qk weight m-tiles: mt0=q-hp0, mt1=q-hp1, mt2=k-hp0, mt3=k-hp1
        # out-col within tile = 64*i + dd  (i head-in-pair, dd hd index)
        wq = np.zeros((1024, 4, 128), np.float32)
        bqv = np.zeros((128, 4), np.float32)
        for mt in range(4):
            t, hp = divmod(mt, 2)       # t: 0=q, 1=k
            for i in range(2):
                h = heads[2 * hp + i]
                cols = h * 192 + 64 * t + np.arange(64)
                wq[:, mt, 64 * i:64 * i + 64] = W_qkv[:, cols]
                bqv[64 * i:64 * i + 64, mt] = b_qkv[cols]
        wq = wq.reshape(4, 2, 128, 4, 128).transpose(2, 0, 1, 3, 4)
        wq8 = wq.astype(E4)
        wq8l = ((wq - wq8.astype(np.float32)) * 8.0).astype(E4)
        wq8s = (wq * 0.125).astype(E4)

        # v weights: col = 64*h + dd
        wv = np.zeros((1024, 256), np.float32)
        bvv = np.zeros((4, 64), np.float32)
        for hh in range(4):
            cols = heads[hh] * 192 + 128 + np.arange(64)
            wv[:, 64 * hh:64 * hh + 64] = W_qkv[:, cols]
            bvv[hh] = b_qkv[cols]
        wv = wv.reshape(4, 2, 128, 256).transpose(2, 0, 1, 3)
        wv8 = wv.astype(E4)
        wv8l = ((wv - wv8.astype(np.float32)) * 8.0).astype(E4)
        wv8s = (wv * 0.125).astype(E4)
        bv2 = np.broadcast_to(bvv[None], (128, 4, 64))

        wo = W_out[g * 256:(g + 1) * 256, :]             # [256, 1024]
        wob = wo.reshape(2, 128, D).transpose(1, 0, 2).astype(BF)

        in_maps.append({
            "xq8": x8,
            "xq8l": x8l,
            "xs8": xs8_a,
            "zro": np.zeros((128, 384), np.float32),
            "wq8": np.ascontiguousarray(wq8),
            "wq8l": np.ascontiguousarray(wq8l),
            "wq8s": np.ascontiguousarray(wq8s),
            "wv8": np.ascontiguousarray(wv8),
            "wv8l": np.ascontiguousarray(wv8l),
            "wv8s": np.ascontiguousarray(wv8s),
            "woutb": np.ascontiguousarray(wob),
            "bq": np.ascontiguousarray(bqv),
            "bv": np.ascontiguousarray(bv2),
            "vone": vone,
            "cmask": np.ascontiguousarray(cmask),
            "identb": identb,
        })

    _CACHE["in_maps"] = in_maps
    res = bass_utils.run_bass_kernel_spmd(nc, in_maps, core_ids=list(range(N_CORES)))

    out = np.zeros((B, S, D), np.float32)
    for c in range(N_CORES):
        b = c // 4
        oT = np.asarray(res.results[c]["outT"]).astype(np.float32)
        out[b] += oT.transpose(1, 0, 2).reshape(D, S).T
    out += b_out
    return out


# revision 3
# speedup vs baseline: 1.0166x; 1.0166x over previous
"""Multi-head causal attention (B=2, S=2048, D=1024, H=16) on 8 TRN2 NeuronCores.
135.3us TimelineSim (baseline 184.4us).

Sharding: core c handles batch b = c // 4 and local head group g = c % 4
(global heads 4g..4g+3).  Each core computes its heads' QKV projections,
causal attention, and a partial output projection; host sums the 4 partials
per batch and adds b_out.

v2 design (vs baseline):
  - QK projection: fp8e4 DoubleRow matmuls, 3-pass hi/lo error compensation
    (x8@W8 + x8lo@W8 + x8@W8lo) -> q,k accurate to ~0.4%; f32r scores.
  - V projection: same 3-pass fp8 DR -> vn in bf16 (k-major, +ones col).
  - Scores: f32r, two heads packed per 128-partition psum tile, causal
    trimming at 256 granularity (f32r needs moving dim >= 256).
  - exp on ACT -> bf16 E tiles; triangle masks on DVE (bf16 2x mode).
  - AV transposed: out[q(128), 65] = E_block[k,q].T @ Vn[k, 65]; moving dim
    is only 65 cols -> ~2x fewer PE cycles than value-major AV.  Ones column
    of Vn gives the softmax denominator per q ON THE PARTITION, so
    normalization is a per-partition reciprocal + tensor_scalar multiply
    (no cross-partition broadcast needed at all).
  - values transposed back to [d, q] with PE transpose matmuls (bf16),
    bf16 output projection.
"""

from contextlib import ExitStack

import numpy as np
import ml_dtypes

import concourse.bass as bass
import concourse.mybir as mybir
import concourse.tile as tile
from concourse import bass_utils

F32 = mybir.dt.float32
F32R = mybir.dt.float32r
BF16 = mybir.dt.bfloat16
FP8 = mybir.dt.float8e4
EXP = mybir.ActivationFunctionType.Exp
COPY = mybir.ActivationFunctionType.Copy
DR = mybir.MatmulPerfMode.DoubleRow

E4 = ml_dtypes.float8_e4m3
BF = ml_dtypes.bfloat16

B, S, D, H = 2, 2048, 1024, 16
HD = D // H          # 64
HL = 4               # heads per core
N_CORES = 8
SC = S // 512        # 4 q-chunks of 512
KT = S // 128        # 16 k-tiles of 128

_CACHE = {}


def _round_f32r(x: np.ndarray) -> np.ndarray:
    """Round f32 to fp32r (11-bit mantissa, RNE) on host."""
    u = np.ascontiguousarray(x, dtype=np.float32).view(np.uint32)
    frac = u & np.uint32(0x00000FFF)
    base = u & np.uint32(0xFFFFF000)
    bit = np.uint32(0x00000800)
    lsb = np.uint32(0x00001000)
    roundup = (frac > bit) | ((frac == bit) & ((u & lsb) != 0))
    return np.where(roundup, base + lsb, base).view(np.float32)


_NO_HOIST = {
    "AllEngineBarrier",
    "EventSemaphore",
    "UnconditionalBranch",
    "CompareAndBranch",
    "BranchHint",
    "IndirectBranch",
    "Halt",
    "Call",
    "OverlayCall",
    "NoOp",
}


def _fix_sync_waits(nc):
    """walrus codegen holds only one sync-wait per engine instruction; hoist
    excess waits onto same-engine NoOps inserted right before."""
    for fn in nc.m.functions:
        for blk in fn.blocks:
            insts = blk.instructions
            out = []
            changed = False
            for inst in insts:
                si = inst.sync_info
                if si is not None and inst.opcode not in _NO_HOIST:
                    waits = list(si.on_wait)
                    if len(waits) > 1:
                        for j, w in enumerate(waits[:-1]):
                            nop = mybir.InstNoOp(name=f"{inst.name}-wfix{j}")
                            nop.engine = inst.engine
                            nop.sync_info = mybir.SyncInfo(on_wait=[w], on_update=[])
                            out.append(nop)
                        inst.sync_info = mybir.SyncInfo(
                            on_wait=[waits[-1]], on_update=list(si.on_update)
                        )
                        changed = True
                out.append(inst)
            if changed:
                blk.instructions = out


def _build(fix_waits=True, dbg=False):
    nc = bass.Bass("TRN2", target_bir_lowering=False, debug=False,
                   num_devices=N_CORES)
    if dbg:
        d_qT = nc.dram_tensor("d_qT", [128, 2, S], F32R, kind="ExternalOutput").ap()
        d_kT = nc.dram_tensor("d_kT", [128, 2, S], F32R, kind="ExternalOutput").ap()
        d_vn = nc.dram_tensor("d_vn", [128, KT, 4, 65], BF16,
                              kind="ExternalOutput").ap()
        d_e = nc.dram_tensor("d_e", [128, 2, 512], BF16, kind="ExternalOutput").ap()
        d_vst = nc.dram_tensor("d_vst", [128, 4, 4, 64], BF16,
                               kind="ExternalOutput").ap()
        d_vnT = nc.dram_tensor("d_vnT", [128, 2, S], BF16,
                               kind="ExternalOutput").ap()

    xq8 = nc.dram_tensor("xq8", [128, 4, 2, S], FP8, kind="ExternalInput").ap()
    xq8l = nc.dram_tensor("xq8l", [128, 4, 2, S], FP8, kind="ExternalInput").ap()
    xs8 = nc.dram_tensor("xs8", [128, 4, 2, S], FP8, kind="ExternalInput").ap()
    zro = nc.dram_tensor("zro", [128, 384], F32R, kind="ExternalInput").ap()
    wq8 = nc.dram_tensor("wq8", [128, 4, 2, 4, 128], FP8, kind="ExternalInput").ap()
    wq8l = nc.dram_tensor("wq8l", [128, 4, 2, 4, 128], FP8, kind="ExternalInput").ap()
    wq8s = nc.dram_tensor("wq8s", [128, 4, 2, 4, 128], FP8, kind="ExternalInput").ap()
    wv8 = nc.dram_tensor("wv8", [128, 4, 2, 256], FP8, kind="ExternalInput").ap()
    wv8l = nc.dram_tensor("wv8l", [128, 4, 2, 256], FP8, kind="ExternalInput").ap()
    wv8s = nc.dram_tensor("wv8s", [128, 4, 2, 256], FP8, kind="ExternalInput").ap()
    woutb = nc.dram_tensor("woutb", [128, 2, D], BF16, kind="ExternalInput").ap()
    bq = nc.dram_tensor("bq", [128, 4], F32, kind="ExternalInput").ap()
    bv = nc.dram_tensor("bv", [128, 4, 64], F32, kind="ExternalInput").ap()
    vone = nc.dram_tensor("vone", [128, KT, 4, 1], BF16, kind="ExternalInput").ap()
    cmask = nc.dram_tensor("cmask", [128, 128], BF16, kind="ExternalInput").ap()
    identb = nc.dram_tensor("identb", [128, 128], BF16, kind="ExternalInput").ap()
    outT = nc.dram_tensor("outT", [128, 8, S], BF16, kind="ExternalOutput").ap()

    with tile.TileContext(nc) as tc, ExitStack() as ctx:
        persist = ctx.enter_context(tc.tile_pool(name="persist", bufs=1))
        xpool = ctx.enter_context(tc.tile_pool(name="xp", bufs=3))
        epool = ctx.enter_context(tc.tile_pool(name="ep", bufs=8))
        spool = ctx.enter_context(tc.tile_pool(name="stp", bufs=3))
        opool = ctx.enter_context(tc.tile_pool(name="op", bufs=6))
        # psum (8 banks): sp 2x2-bank, po 1x2-bank, small (pq/pv/pu/tr) 2x1
        ps = ctx.enter_context(tc.tile_pool(name="ps", bufs=2, space="PSUM"))

        wq_sb = persist.tile([128, 4, 2, 4, 128], FP8, tag="wq")
        wql_sb = persist.tile([128, 4, 2, 4, 128], FP8, tag="wql")
        wqs_sb = persist.tile([128, 4, 2, 4, 128], FP8, tag="wqs")
        wv_sb = persist.tile([128, 4, 2, 256], FP8, tag="wv")
        wvl_sb = persist.tile([128, 4, 2, 256], FP8, tag="wvl")
        wvs_sb = persist.tile([128, 4, 2, 256], FP8, tag="wvs")
        zro_sb = persist.tile([128, 384], F32R, tag="zro")
        wo_sb = persist.tile([128, 2, D], BF16, tag="wo")
        bq_sb = persist.tile([128, 4], F32, tag="bq")
        bv_sb = persist.tile([128, 4, 64], F32, tag="bv")
        cm_sb = persist.tile([128, 128], BF16, tag="cm")
        id_sb = persist.tile([128, 128], BF16, tag="id")
        qT = persist.tile([128, 2, S], F32R, tag="qT")
        kT = persist.tile([128, 2, S], F32R, tag="kT")
        vn = persist.tile([128, KT, 4, 65], BF16, tag="vn")
        vnT = persist.tile([128, 2, S], BF16, tag="vnT")

        # first x chunk first so the first matmuls start early
        xc0 = xpool.tile([128, 4, 2, 512], FP8, tag="xc", name="xc0")
        xl0 = xpool.tile([128, 4, 2, 512], FP8, tag="xl", name="xl0")
        xs0 = xpool.tile([128, 4, 2, 512], FP8, tag="xs", name="xs0")
        nc.sync.dma_start(xc0[:], xq8[:, :, :, 0:512])
        nc.scalar.dma_start(bq_sb[:], bq)
        nc.scalar.dma_start(wq_sb[:], wq8)
        nc.sync.dma_start(xs0[:], xs8[:, :, :, 0:512])
        nc.scalar.dma_start(wql_sb[:], wq8l)
        nc.sync.dma_start(xl0[:], xq8l[:, :, :, 0:512])
        nc.scalar.dma_start(wqs_sb[:], wq8s)
        nc.scalar.dma_start(bv_sb[:], bv)
        nc.scalar.dma_start(wv_sb[:], wv8)
        nc.scalar.dma_start(wvl_sb[:], wv8l)
        nc.scalar.dma_start(wvs_sb[:], wv8s)
        nc.scalar.dma_start(cm_sb[:], cmask)
        nc.scalar.dma_start(zro_sb[:], zro)
        nc.scalar.dma_start(id_sb[:], identb)
        nc.scalar.dma_start(wo_sb[:], woutb)
        # ones column of vn (softmax denominators) via memset, not DMA
        # (a strided single-element-column DMA costs ~3.6us of DMA engines)
        nc.vector.memset(vn[:, :, :, 64:65], 1.0)
        xtiles = {0: (xc0, xl0, xs0)}

        def qkv_dma(qc):
            qs = slice(qc * 512, (qc + 1) * 512)
            xc = xpool.tile([128, 4, 2, 512], FP8, tag="xc", name=f"xc{qc}")
            xl = xpool.tile([128, 4, 2, 512], FP8, tag="xl", name=f"xl{qc}")
            xs = xpool.tile([128, 4, 2, 512], FP8, tag="xs", name=f"xs{qc}")
            nc.sync.dma_start(xc[:], xq8[:, :, :, qs])
            nc.sync.dma_start(xl[:], xq8l[:, :, :, qs])
            nc.sync.dma_start(xs[:], xs8[:, :, :, qs])
            xtiles[qc] = (xc, xl, xs)

        def qk_tile(qc, mt):
            qs = slice(qc * 512, (qc + 1) * 512)
            xc, xl, xs = xtiles[qc]
            pq = ps.tile([128, 512], F32, tag="q1", name=f"pq{qc}{mt}")
            passes = [(wq_sb, xc), (wql_sb, xs), (wqs_sb, xl)]
            i = 0
            for wsb, xsb in passes:
                for kp in range(4):
                    nc.tensor.matmul(
                        pq[:], wsb[:, kp, :, mt, :], xsb[:, kp, :, :],
                        start=(i == 0), stop=(i == 11), perf_mode=DR)
                    i += 1
            dst = (qT if mt < 2 else kT)[:, mt % 2, qs]
            nc.vector.tensor_scalar_add(dst, pq[:], bq_sb[:, mt:mt + 1])

        def v_tile(qc, j):
            st = 4 * qc + j
            xc, xl, xs = xtiles[qc]
            pv = ps.tile([128, 512], F32, tag="q1", name=f"pv{qc}{j}")
            passes = [(wv_sb, xc), (wvl_sb, xs), (wvs_sb, xl)]
            i = 0
            for wsb, xsb in passes:
                for kp in range(4):
                    nc.tensor.matmul(
                        pv[0:128, 0:256], xsb[:, kp, :, j * 128:(j + 1) * 128],
                        wsb[:, kp, :, :],
                        start=(i == 0), stop=(i == 11), perf_mode=DR)
                    i += 1
            nc.vector.tensor_add(
                vn[:, st, :, 0:64],
                pv[0:128, 0:256].rearrange("p (h d) -> p h d", h=4),
                bv_sb[:])

        def qkv_pieces(qc):
            ps_ = [lambda qc=qc: qkv_dma(qc)] if qc > 0 else []
            for mt in (0, 2):
                ps_.append(lambda qc=qc, mt=mt: qk_tile(qc, mt))
            for j in range(4):
                ps_.append(lambda qc=qc, j=j: v_tile(qc, j))
            for mt in (1, 3):
                ps_.append(lambda qc=qc, mt=mt: qk_tile(qc, mt))
            return ps_

        vst_tiles = {}

        def tr_piece(qc, qt):
            vst = vst_tiles[qc]
            for dh in range(2):
                ptr = ps.tile([128, 128], BF16, tag="q1", name=f"tr{qc}{qt}{dh}")
                nc.tensor.matmul(ptr[:], vst[:, qt, 2 * dh:2 * dh + 2, :],
                                 id_sb[:], is_transpose=True)
                nc.vector.tensor_copy(
                    vnT[:, dh, qc * 512 + qt * 128:qc * 512 + (qt + 1) * 128],
                    ptr[:])

        def op_piece(qc, m):
            qs = slice(qc * 512, (qc + 1) * 512)
            pu = ps.tile([128, 512], F32, tag="q1", name=f"pu{qc}{m}")
            for t in range(2):
                nc.tensor.matmul(pu[:], wo_sb[:, t, m * 128:(m + 1) * 128],
                                 vnT[:, t, qs], start=(t == 0), stop=(t == 1))
            ou = opool.tile([128, 512], BF16, tag="ou", name=f"ou{qc}{m}")
            if qc == SC - 1 and m % 2 == 1:
                nc.scalar.copy(ou[:], pu[:])   # tail: ACT is idle
            else:
                nc.vector.tensor_copy(ou[:], pu[:])
            nc.sync.dma_start(outT[:, m, qs], ou[:])

        def post_pieces(qc):
            ps_ = [lambda qc=qc, qt=qt: tr_piece(qc, qt) for qt in range(4)]
            ps_ += [lambda qc=qc, m=m: op_piece(qc, m) for m in range(8)]
            return ps_

        def attn_qc(qc, queue):
            """Emit attention for chunk qc, interleaving `queue` pieces (PE
            work for the next chunk's projections and the previous chunk's
            transposes/output projection) between ki steps so every engine
            stays fed while the exp (ACT) chain runs."""
            vst = spool.tile([128, 4, 4, 64], BF16, tag="vst", name=f"vs{qc}")
            vst_tiles[qc] = vst
            n_ki = 4 * qc + 4
            n_steps = 2 * n_ki + 2
            qi = 0
            emitted = 0.0

            def drain(frac):
                nonlocal qi, emitted
                emitted += frac
                while qi < len(queue) and qi < emitted:
                    queue[qi]()
                    qi += 1

            per_step = len(queue) / n_steps
            for hp in range(2):
                po = ps.tile([128, 2, 512], F32, tag="po", name=f"po{qc}{hp}", bufs=1)
                for i in range(2):
                    # one start=True matmul zeroes all four qt accumulation
                    # regions of this bank (psum pending-zero is bank-wide)
                    nc.tensor.matmul(po[:, i, 0:260], zro_sb[0:1, 0:128],
                                     zro_sb[0:1, 0:260], start=True, stop=False,
                                     skip_group_check=True)
                for ki in range(n_ki):
                    j = ki - 4 * qc  # >= 0 on diagonal tiles
                    o_exp = max(0, 128 * j)
                    o_sc = min(o_exp, 256)  # f32r moving dim must be >= 256
                    sp = ps.tile([128, 2, 512], F32, tag="s",
                                 name=f"sp{qc}{hp}{ki}")
                    for i in range(2):
                        vp = 64 * i
                        nc.tensor.matmul(
                            sp[:, i, o_sc:512],
                            kT[vp:vp + 64, hp, ki * 128:(ki + 1) * 128],
                            qT[vp:vp + 64, hp, qc * 512 + o_sc:(qc + 1) * 512],
                            start=True, stop=True, tile_position=(vp, 0))
                    e = epool.tile([128, 2, 512], BF16, tag="e",
                                   name=f"e{qc}{hp}{ki}")
                    nc.scalar.activation(e[:, :, o_exp:512], sp[:, :, o_exp:512],
                                         EXP, scale=0.125)
                    if j >= 0:  # diagonal: mask the [128,128] triangle block
                        for i in range(2):
                            es = e[:, i, o_exp:o_exp + 128]
                            nc.vector.tensor_mul(es, es, cm_sb[:])
                    # AV transposed: po[q, 65] += E_block.T @ Vn
                    for i in range(2):
                        for qt in range(max(0, j), 4):
                            nc.tensor.matmul(
                                po[:, i, qt * 65:qt * 65 + 65],
                                e[:, i, qt * 128:(qt + 1) * 128],
                                vn[:, ki, 2 * hp + i, :],
                                start=False, stop=(ki == 4 * qc + qt),
                                skip_group_check=True)
                    drain(per_step)
                # normalize: per-partition recip of denominators, then scale
                rc = spool.tile([128, 2, 4], F32, tag="rc", name=f"rc{qc}{hp}")
                for i in range(2):
                    dn = po[:, i, 0:260].rearrange("p (qt c) -> p qt c", c=65)
                    with nc.allow_low_precision(reason="softmax recip"):
                        nc.vector.reciprocal(rc[:, i, :], dn[:, 0:4, 64:65])
                    for qt in range(4):
                        nc.vector.tensor_scalar_mul(
                            vst[:, qt, 2 * hp + i, :],
                            po[:, i, qt * 65:qt * 65 + 64],
                            rc[:, i, qt:qt + 1])
                drain(1.0)
            drain(len(queue))

        for piece in qkv_pieces(0):
            piece()
        for c in range(SC):
            queue = []
            if c >= 1:
                queue += post_pieces(c - 1)
            if c + 1 < SC:
                queue += qkv_pieces(c + 1)
            attn_qc(c, queue)
        for piece in post_pieces(SC - 1):
            piece()
        if dbg:
            nc.sync.dma_start(d_vst, vst_tiles[0][:])
            nc.sync.dma_start(d_qT, qT[:])
            nc.sync.dma_start(d_kT, kT[:])
            nc.sync.dma_start(d_vn, vn[:])
            nc.sync.dma_start(d_vnT, vnT[:])

    if fix_waits:
        _fix_sync_waits(nc)
    return nc


def _get_nc():
    if "nc" not in _CACHE:
        _CACHE["nc"] = _build()
    return _CACHE["nc"]


def _dr_layout(xb):
    """[S, 1024] -> [128, 4, 2, S]: p=partition, kp=k-tile-pair, sl=slot."""
    return np.ascontiguousarray(
        xb.T.reshape(4, 2, 128, xb.shape[0]).transpose(2, 0, 1, 3))


def kernel(x, W_qkv, b_qkv, W_out, b_out):
    x = np.asarray(x, np.float32)
    W_qkv = np.asarray(W_qkv, np.float32)
    b_qkv = np.asarray(b_qkv, np.float32)
    W_out = np.asarray(W_out, np.float32)
    b_out = np.asarray(b_out, np.float32)

    nc = _get_nc()

    kk = np.arange(128)[:, None]
    qq = np.arange(128)[None, :]
    cmask = (kk <= qq).astype(BF)
    identb = np.eye(128, dtype=np.float32).astype(BF)
    vone = np.ones((128, KT, 4, 1), np.float32).astype(BF)

    in_maps = []
    for c in range(N_CORES):
        b, g = divmod(c, 4)
        heads = [4 * g + i for i in range(HL)]

        xb = x[b]                                        # [S, 1024]
        xr = _dr_layout(xb)
        x8 = xr.astype(E4)
        x8l = ((xr - x8.astype(np.float32)) * 8.0).astype(E4)
        xs8_a = (xr * 0.125).astype(E4)

        # qk weight m-tiles: mt0=q-hp0, mt1=q-hp1, mt2=k-hp0, mt3=k-hp1
        # out-col within tile = 64*i + dd  (i head-in-pair, dd hd index)
        wq = np.zeros((1024, 4, 128), np.float32)
        bqv = np.zeros((128, 4), np.float32)
        for mt in range(4):
            t, hp = divmod(mt, 2)       # t: 0=q, 1=k
            for i in range(2):
                h = heads[2 * hp + i]
                cols = h * 192 + 64 * t + np.arange(64)
                wq[:, mt, 64 * i:64 * i + 64] = W_qkv[:, cols]
                bqv[64 * i:64 * i + 64, mt] = b_qkv[cols]
        wq = wq.reshape(4, 2, 128, 4, 128).transpose(2, 0, 1, 3, 4)
        wq8 = wq.astype(E4)
        wq8l = ((wq - wq8.astype(np.float32)) * 8.0).astype(E4)
        wq8s = (wq * 0.125).astype(E4)

        # v weights: col = 64*h + dd
        wv = np.zeros((1024, 256), np.float32)
        bvv = np.zeros((4, 64), np.float32)
        for hh in range(4):
            cols = heads[hh] * 192 + 128 + np.arange(64)
            wv[:, 64 * hh:64 * hh + 64] = W_qkv[:, cols]
            bvv[hh] = b_qkv[cols]
        wv = wv.reshape(4, 2, 128, 256).transpose(2, 0, 1, 3)
        wv8 = wv.astype(E4)
        wv8l = ((wv - wv8.astype(np.float32)) * 8.0).astype(E4)
        wv8s = (wv * 0.125).astype(E4)
        bv2 = np.broadcast_to(bvv[None], (128, 4, 64))

        wo = W_out[g * 256:(g + 1) * 256, :]             # [256, 1024]
        wob = wo.reshape(2, 128, D).transpose(1, 0, 2).astype(BF)

        in_maps.append({
            "xq8": x8,
            "xq8l": x8l,
            "xs8": xs8_a,
            "zro": np.zeros((128, 384), np.float32),
            "wq8": np.ascontiguousarray(wq8),
            "wq8l": np.ascontiguousarray(wq8l),
            "wq8s": np.ascontiguousarray(wq8s),
            "wv8": np.ascontiguousarray(wv8),
            "wv8l": np.ascontiguousarray(wv8l),
            "wv8s": np.ascontiguousarray(wv8s),
            "woutb": np.ascontiguousarray(wob),
            "bq": np.ascontiguousarray(bqv),
            "bv": np.ascontiguousarray(bv2),
            "vone": vone,
            "cmask": np.ascontiguousarray(cmask),
            "identb": identb,
        })

    _CACHE["in_maps"] = in_maps
    res = bass_utils.run_bass_kernel_spmd(nc, in_maps, core_ids=list(range(N_CORES)))

    out = np.zeros((B, S, D), np.float32)
    for c in range(N_CORES):
        b = c // 4
        oT = np.asarray(res.results[c]["outT"]).astype(np.float32)
        out[b] += oT.transpose(1, 0, 2).reshape(D, S).T
    out += b_out
    return out


# revision 4
# speedup vs baseline: 1.0281x; 1.0113x over previous
"""Multi-head causal attention (B=2, S=2048, D=1024, H=16) on 8 TRN2 NeuronCores.
135.3us TimelineSim (baseline 184.4us).

Sharding: core c handles batch b = c // 4 and local head group g = c % 4
(global heads 4g..4g+3).  Each core computes its heads' QKV projections,
causal attention, and a partial output projection; host sums the 4 partials
per batch and adds b_out.

v2 design (vs baseline):
  - QK projection: fp8e4 DoubleRow matmuls, 3-pass hi/lo error compensation
    (x8@W8 + x8lo@W8 + x8@W8lo) -> q,k accurate to ~0.4%; f32r scores.
  - V projection: same 3-pass fp8 DR -> vn in bf16 (k-major, +ones col).
  - Scores: f32r, two heads packed per 128-partition psum tile, causal
    trimming at 256 granularity (f32r needs moving dim >= 256).
  - exp on ACT -> bf16 E tiles; triangle masks on DVE (bf16 2x mode).
  - AV transposed: out[q(128), 65] = E_block[k,q].T @ Vn[k, 65]; moving dim
    is only 65 cols -> ~2x fewer PE cycles than value-major AV.  Ones column
    of Vn gives the softmax denominator per q ON THE PARTITION, so
    normalization is a per-partition reciprocal + tensor_scalar multiply
    (no cross-partition broadcast needed at all).
  - values transposed back to [d, q] with PE transpose matmuls (bf16),
    bf16 output projection.
"""

from contextlib import ExitStack

import numpy as np
import ml_dtypes

import concourse.bass as bass
import concourse.mybir as mybir
import concourse.tile as tile
from concourse import bass_utils

F32 = mybir.dt.float32
F32R = mybir.dt.float32r
BF16 = mybir.dt.bfloat16
FP8 = mybir.dt.float8e4
EXP = mybir.ActivationFunctionType.Exp
COPY = mybir.ActivationFunctionType.Copy
DR = mybir.MatmulPerfMode.DoubleRow

E4 = ml_dtypes.float8_e4m3
BF = ml_dtypes.bfloat16

B, S, D, H = 2, 2048, 1024, 16
HD = D // H          # 64
HL = 4               # heads per core
N_CORES = 8
SC = S // 512        # 4 q-chunks of 512
KT = S // 128        # 16 k-tiles of 128

_CACHE = {}


def _round_f32r(x: np.ndarray) -> np.ndarray:
    """Round f32 to fp32r (11-bit mantissa, RNE) on host."""
    u = np.ascontiguousarray(x, dtype=np.float32).view(np.uint32)
    frac = u & np.uint32(0x00000FFF)
    base = u & np.uint32(0xFFFFF000)
    bit = np.uint32(0x00000800)
    lsb = np.uint32(0x00001000)
    roundup = (frac > bit) | ((frac == bit) & ((u & lsb) != 0))
    return np.where(roundup, base + lsb, base).view(np.float32)


_NO_HOIST = {
    "AllEngineBarrier",
    "EventSemaphore",
    "UnconditionalBranch",
    "CompareAndBranch",
    "BranchHint",
    "IndirectBranch",
    "Halt",
    "Call",
    "OverlayCall",
    "NoOp",
}


def _fix_sync_waits(nc):
    """walrus codegen holds only one sync-wait per engine instruction; hoist
    excess waits onto same-engine NoOps inserted right before."""
    for fn in nc.m.functions:
        for blk in fn.blocks:
            insts = blk.instructions
            out = []
            changed = False
            for inst in insts:
                si = inst.sync_info
                if si is not None and inst.opcode not in _NO_HOIST:
                    waits = list(si.on_wait)
                    if len(waits) > 1:
                        for j, w in enumerate(waits[:-1]):
                            nop = mybir.InstNoOp(name=f"{inst.name}-wfix{j}")
                            nop.engine = inst.engine
                            nop.sync_info = mybir.SyncInfo(on_wait=[w], on_update=[])
                            out.append(nop)
                        inst.sync_info = mybir.SyncInfo(
                            on_wait=[waits[-1]], on_update=list(si.on_update)
                        )
                        changed = True
                out.append(inst)
            if changed:
                blk.instructions = out


def _build(fix_waits=True, dbg=False):
    nc = bass.Bass("TRN2", target_bir_lowering=False, debug=False,
                   num_devices=N_CORES)
    if dbg:
        d_qT = nc.dram_tensor("d_qT", [128, 2, S], F32R, kind="ExternalOutput").ap()
        d_kT = nc.dram_tensor("d_kT", [128, 2, S], F32R, kind="ExternalOutput").ap()
        d_vn = nc.dram_tensor("d_vn", [128, KT, 4, 65], BF16,
                              kind="ExternalOutput").ap()
        d_e = nc.dram_tensor("d_e", [128, 2, 512], BF16, kind="ExternalOutput").ap()
        d_vst = nc.dram_tensor("d_vst", [128, 4, 4, 64], BF16,
                               kind="ExternalOutput").ap()
        d_vnT = nc.dram_tensor("d_vnT", [128, 2, S], BF16,
                               kind="ExternalOutput").ap()

    xq8 = nc.dram_tensor("xq8", [128, 4, 2, S], FP8, kind="ExternalInput").ap()
    xq8l = nc.dram_tensor("xq8l", [128, 4, 2, S], FP8, kind="ExternalInput").ap()
    xs8 = nc.dram_tensor("xs8", [128, 4, 2, S], FP8, kind="ExternalInput").ap()
    zro = nc.dram_tensor("zro", [128, 384], F32R, kind="ExternalInput").ap()
    wq8 = nc.dram_tensor("wq8", [128, 4, 2, 4, 128], FP8, kind="ExternalInput").ap()
    wq8l = nc.dram_tensor("wq8l", [128, 4, 2, 4, 128], FP8, kind="ExternalInput").ap()
    wq8s = nc.dram_tensor("wq8s", [128, 4, 2, 4, 128], FP8, kind="ExternalInput").ap()
    wv8 = nc.dram_tensor("wv8", [128, 4, 2, 256], FP8, kind="ExternalInput").ap()
    wv8l = nc.dram_tensor("wv8l", [128, 4, 2, 256], FP8, kind="ExternalInput").ap()
    wv8s = nc.dram_tensor("wv8s", [128, 4, 2, 256], FP8, kind="ExternalInput").ap()
    woutb = nc.dram_tensor("woutb", [128, 2, D], BF16, kind="ExternalInput").ap()
    bq = nc.dram_tensor("bq", [128, 4], F32, kind="ExternalInput").ap()
    bv = nc.dram_tensor("bv", [128, 4, 64], F32, kind="ExternalInput").ap()
    vone = nc.dram_tensor("vone", [128, KT, 4, 1], BF16, kind="ExternalInput").ap()
    cmask = nc.dram_tensor("cmask", [128, 128], BF16, kind="ExternalInput").ap()
    identb = nc.dram_tensor("identb", [128, 128], BF16, kind="ExternalInput").ap()
    outT = nc.dram_tensor("outT", [128, 8, S], BF16, kind="ExternalOutput").ap()

    with tile.TileContext(nc) as tc, ExitStack() as ctx:
        persist = ctx.enter_context(tc.tile_pool(name="persist", bufs=1))
        xpool = ctx.enter_context(tc.tile_pool(name="xp", bufs=3))
        epool = ctx.enter_context(tc.tile_pool(name="ep", bufs=8))
        spool = ctx.enter_context(tc.tile_pool(name="stp", bufs=3))
        opool = ctx.enter_context(tc.tile_pool(name="op", bufs=6))
        # psum (8 banks): sp 2x2-bank, po 1x2-bank, small (pq/pv/pu/tr) 2x1
        ps = ctx.enter_context(tc.tile_pool(name="ps", bufs=2, space="PSUM"))

        wq_sb = persist.tile([128, 4, 2, 4, 128], FP8, tag="wq")
        wql_sb = persist.tile([128, 4, 2, 4, 128], FP8, tag="wql")
        wqs_sb = persist.tile([128, 4, 2, 4, 128], FP8, tag="wqs")
        wv_sb = persist.tile([128, 4, 2, 256], FP8, tag="wv")
        wvl_sb = persist.tile([128, 4, 2, 256], FP8, tag="wvl")
        wvs_sb = persist.tile([128, 4, 2, 256], FP8, tag="wvs")
        zro_sb = persist.tile([128, 384], F32R, tag="zro")
        wo_sb = persist.tile([128, 2, D], BF16, tag="wo")
        bq_sb = persist.tile([128, 4], F32, tag="bq")
        bv_sb = persist.tile([128, 4, 64], F32, tag="bv")
        cm_sb = persist.tile([128, 128], BF16, tag="cm")
        id_sb = persist.tile([128, 128], BF16, tag="id")
        qT = persist.tile([128, 2, S], F32R, tag="qT")
        kT = persist.tile([128, 2, S], F32R, tag="kT")
        vn = persist.tile([128, KT, 4, 65], BF16, tag="vn")
        vnT = persist.tile([128, 2, S], BF16, tag="vnT")

        # first x chunk first so the first matmuls start early
        xc0 = xpool.tile([128, 4, 2, 512], FP8, tag="xc", name="xc0")
        xl0 = xpool.tile([128, 4, 2, 512], FP8, tag="xl", name="xl0")
        xs0 = xpool.tile([128, 4, 2, 512], FP8, tag="xs", name="xs0")
        nc.sync.dma_start(xc0[:], xq8[:, :, :, 0:512])
        nc.scalar.dma_start(bq_sb[:], bq)
        nc.scalar.dma_start(wq_sb[:], wq8)
        nc.sync.dma_start(xs0[:], xs8[:, :, :, 0:512])
        nc.scalar.dma_start(wql_sb[:], wq8l)
        nc.sync.dma_start(xl0[:], xq8l[:, :, :, 0:512])
        nc.scalar.dma_start(wqs_sb[:], wq8s)
        nc.scalar.dma_start(bv_sb[:], bv)
        nc.scalar.dma_start(wv_sb[:], wv8)
        nc.scalar.dma_start(wvl_sb[:], wv8l)
        nc.scalar.dma_start(wvs_sb[:], wv8s)
        nc.scalar.dma_start(cm_sb[:], cmask)
        nc.scalar.dma_start(zro_sb[:], zro)
        nc.scalar.dma_start(id_sb[:], identb)
        nc.scalar.dma_start(wo_sb[:], woutb)
        # ones column of vn (softmax denominators) via memset, not DMA
        # (a strided single-element-column DMA costs ~3.6us of DMA engines)
        nc.vector.memset(vn[:, :, :, 64:65], 1.0)
        xtiles = {0: (xc0, xl0, xs0)}

        def qkv_dma(qc):
            qs = slice(qc * 512, (qc + 1) * 512)
            xc = xpool.tile([128, 4, 2, 512], FP8, tag="xc", name=f"xc{qc}")
            xl = xpool.tile([128, 4, 2, 512], FP8, tag="xl", name=f"xl{qc}")
            xs = xpool.tile([128, 4, 2, 512], FP8, tag="xs", name=f"xs{qc}")
            nc.sync.dma_start(xc[:], xq8[:, :, :, qs])
            nc.sync.dma_start(xl[:], xq8l[:, :, :, qs])
            nc.sync.dma_start(xs[:], xs8[:, :, :, qs])
            xtiles[qc] = (xc, xl, xs)

        def qk_tile(qc, mt):
            qs = slice(qc * 512, (qc + 1) * 512)
            xc, xl, xs = xtiles[qc]
            pq = ps.tile([128, 512], F32, tag="q1", name=f"pq{qc}{mt}")
            passes = [(wq_sb, xc), (wql_sb, xs), (wqs_sb, xl)]
            i = 0
            for wsb, xsb in passes:
                for kp in range(4):
                    nc.tensor.matmul(
                        pq[:], wsb[:, kp, :, mt, :], xsb[:, kp, :, :],
                        start=(i == 0), stop=(i == 11), perf_mode=DR)
                    i += 1
            dst = (qT if mt < 2 else kT)[:, mt % 2, qs]
            nc.vector.tensor_scalar_add(dst, pq[:], bq_sb[:, mt:mt + 1])

        def v_tile(qc, j):
            st = 4 * qc + j
            xc, xl, xs = xtiles[qc]
            pv = ps.tile([128, 512], F32, tag="q1", name=f"pv{qc}{j}")
            passes = [(wv_sb, xc), (wvl_sb, xs), (wvs_sb, xl)]
            i = 0
            for wsb, xsb in passes:
                for kp in range(4):
                    nc.tensor.matmul(
                        pv[0:128, 0:256], xsb[:, kp, :, j * 128:(j + 1) * 128],
                        wsb[:, kp, :, :],
                        start=(i == 0), stop=(i == 11), perf_mode=DR)
                    i += 1
            nc.vector.tensor_add(
                vn[:, st, :, 0:64],
                pv[0:128, 0:256].rearrange("p (h d) -> p h d", h=4),
                bv_sb[:])

        def qkv_pieces(qc):
            ps_ = [lambda qc=qc: qkv_dma(qc)] if qc > 0 else []
            for mt in (0, 2):
                ps_.append(lambda qc=qc, mt=mt: qk_tile(qc, mt))
            for j in range(4):
                ps_.append(lambda qc=qc, j=j: v_tile(qc, j))
            for mt in (1, 3):
                ps_.append(lambda qc=qc, mt=mt: qk_tile(qc, mt))
            return ps_

        vst_tiles = {}

        def tr_piece(qc, qt):
            vst = vst_tiles[qc]
            for dh in range(2):
                ptr = ps.tile([128, 128], BF16, tag="q1", name=f"tr{qc}{qt}{dh}")
                nc.tensor.matmul(ptr[:], vst[:, qt, 2 * dh:2 * dh + 2, :],
                                 id_sb[:], is_transpose=True)
                nc.vector.tensor_copy(
                    vnT[:, dh, qc * 512 + qt * 128:qc * 512 + (qt + 1) * 128],
                    ptr[:])

        def op_piece(qc, m):
            qs = slice(qc * 512, (qc + 1) * 512)
            pu = ps.tile([128, 512], F32, tag="q1", name=f"pu{qc}{m}")
            for t in range(2):
                nc.tensor.matmul(pu[:], wo_sb[:, t, m * 128:(m + 1) * 128],
                                 vnT[:, t, qs], start=(t == 0), stop=(t == 1))
            ou = opool.tile([128, 512], BF16, tag="ou", name=f"ou{qc}{m}")
            if qc == SC - 1 and m % 2 == 1:
                nc.scalar.copy(ou[:], pu[:])   # tail: ACT is idle
            else:
                nc.vector.tensor_copy(ou[:], pu[:])
            nc.sync.dma_start(outT[:, m, qs], ou[:])

        def post_pieces(qc):
            ps_ = [lambda qc=qc, qt=qt: tr_piece(qc, qt) for qt in range(4)]
            ps_ += [lambda qc=qc, m=m: op_piece(qc, m) for m in range(8)]
            return ps_

        def attn_qc(qc, queue):
            """Emit attention for chunk qc, interleaving `queue` pieces (PE
            work for the next chunk's projections and the previous chunk's
            transposes/output projection) between ki steps so every engine
            stays fed while the exp (ACT) chain runs."""
            vst = spool.tile([128, 4, 4, 64], BF16, tag="vst", name=f"vs{qc}")
            vst_tiles[qc] = vst
            n_ki = 4 * qc + 4
            n_steps = 2 * n_ki + 2
            qi = 0
            emitted = 0.0

            def drain(frac):
                nonlocal qi, emitted
                emitted += frac
                while qi < len(queue) and qi < emitted:
                    queue[qi]()
                    qi += 1

            per_step = 0.6 * len(queue) / n_steps
            for hp in range(2):
                po = ps.tile([128, 2, 512], F32, tag="po", name=f"po{qc}{hp}", bufs=1)
                for i in range(2):
                    # one start=True matmul zeroes all four qt accumulation
                    # regions of this bank (psum pending-zero is bank-wide)
                    nc.tensor.matmul(po[:, i, 0:260], zro_sb[0:1, 0:128],
                                     zro_sb[0:1, 0:260], start=True, stop=False,
                                     skip_group_check=True)
                for ki in range(n_ki):
                    j = ki - 4 * qc  # >= 0 on diagonal tiles
                    o_exp = max(0, 128 * j)
                    o_sc = min(o_exp, 256)  # f32r moving dim must be >= 256
                    sp = ps.tile([128, 2, 512], F32, tag="s",
                                 name=f"sp{qc}{hp}{ki}")
                    for i in range(2):
                        vp = 64 * i
                        nc.tensor.matmul(
                            sp[:, i, o_sc:512],
                            kT[vp:vp + 64, hp, ki * 128:(ki + 1) * 128],
                            qT[vp:vp + 64, hp, qc * 512 + o_sc:(qc + 1) * 512],
                            start=True, stop=True, tile_position=(vp, 0))
                    e = epool.tile([128, 2, 512], BF16, tag="e",
                                   name=f"e{qc}{hp}{ki}")
                    nc.scalar.activation(e[:, :, o_exp:512], sp[:, :, o_exp:512],
                                         EXP, scale=0.125)
                    if j >= 0:  # diagonal: mask the [128,128] triangle block
                        for i in range(2):
                            es = e[:, i, o_exp:o_exp + 128]
                            nc.vector.tensor_mul(es, es, cm_sb[:])
                    # AV transposed: po[q, 65] += E_block.T @ Vn
                    for i in range(2):
                        for qt in range(max(0, j), 4):
                            nc.tensor.matmul(
                                po[:, i, qt * 65:qt * 65 + 65],
                                e[:, i, qt * 128:(qt + 1) * 128],
                                vn[:, ki, 2 * hp + i, :],
                                start=False, stop=(ki == 4 * qc + qt),
                                skip_group_check=True)
                    drain(per_step)
                # normalize: per-partition recip of denominators, then scale
                rc = spool.tile([128, 2, 4], F32, tag="rc", name=f"rc{qc}{hp}")
                for i in range(2):
                    dn = po[:, i, 0:260].rearrange("p (qt c) -> p qt c", c=65)
                    with nc.allow_low_precision(reason="softmax recip"):
                        nc.vector.reciprocal(rc[:, i, :], dn[:, 0:4, 64:65])
                    for qt in range(4):
                        nc.vector.tensor_scalar_mul(
                            vst[:, qt, 2 * hp + i, :],
                            po[:, i, qt * 65:qt * 65 + 64],
                            rc[:, i, qt:qt + 1])
                drain(1.0)
            drain(len(queue))

        for piece in qkv_pieces(0):
            piece()
        for c in range(SC):
            queue = []
            if c >= 1:
                queue += post_pieces(c - 1)
            if c + 1 < SC:
                queue += qkv_pieces(c + 1)
            attn_qc(c, queue)
        for piece in post_pieces(SC - 1):
            piece()
        if dbg:
            nc.sync.dma_start(d_vst, vst_tiles[0][:])
            nc.sync.dma_start(d_qT, qT[:])
            nc.sync.dma_start(d_kT, kT[:])
            nc.sync.dma_start(d_vn, vn[:])
            nc.sync.dma_start(d_vnT, vnT[:])

    if fix_waits:
        _fix_sync_waits(nc)
    return nc


def _get_nc():
    if "nc" not in _CACHE:
        _CACHE["nc"] = _build()
    return _CACHE["nc"]


def _dr_layout(xb):
    """[S, 1024] -> [128, 4, 2, S]: p=partition, kp=k-tile-pair, sl=slot."""
    return np.ascontiguousarray(
        xb.T.reshape(4, 2, 128, xb.shape[0]).transpose(2, 0, 1, 3))


def kernel(x, W_qkv, b_qkv, W_out, b_out):
    x = np.asarray(x, np.float32)
    W_qkv = np.asarray(W_qkv, np.float32)
    b_qkv = np.asarray(b_qkv, np.float32)
    W_out = np.asarray(W_out, np.float32)
    b_out = np.asarray(b_out, np.float32)

    nc = _get_nc()

    kk = np.arange(128)[:, None]
    qq = np.arange(128)[None, :]
    cmask = (kk <= qq).astype(BF)
    identb = np.eye(128, dtype=np.float32).astype(BF)
    vone = np.ones((128, KT, 4, 1), np.float32).astype(BF)

    in_maps = []
    for c in range(N_CORES):
        b, g = divmod(c, 4)
        heads = [4 * g + i for i in range(HL)]

        xb = x[b]                                        # [S, 1024]
        xr = _dr_layout(xb)
        x8 = xr.astype(E4)
        x8l = ((xr - x8.astype(np.float32)) * 8.0).astype(E4)
        xs8_a = (xr * 0.125).astype(E4)

        # qk weight m-tiles: mt0=q-hp0, mt1=q-hp1, mt2=k-hp0, mt3=k-hp1
        # out-col within tile = 64*i + dd  (i head-in-pair, dd hd index)
        wq = np.zeros((1024, 4, 128), np.float32)
        bqv = np.zeros((128, 4), np.float32)
        for mt in range(4):
            t, hp = divmod(mt, 2)       # t: 0=q, 1=k
            for i in range(2):
                h = heads[2 * hp + i]
                cols = h * 192 + 64 * t + np.arange(64)
                wq[:, mt, 64 * i:64 * i + 64] = W_qkv[:, cols]
                bqv[64 * i:64 * i + 64, mt] = b_qkv[cols]
        wq = wq.reshape(4, 2, 128, 4, 128).transpose(2, 0, 1, 3, 4)
        wq8 = wq.astype(E4)
        wq8l = ((wq - wq8.astype(np.float32)) * 8.0).astype(E4)
        wq8s = (wq * 0.125).astype(E4)

        # v weights: col = 64*h + dd
        wv = np.zeros((1024, 256), np.float32)
        bvv = np.zeros((4, 64), np.float32)
        for hh in range(4):
            cols = heads[hh] * 192 + 128 + np.arange(64)
            wv[:, 64 * hh:64 * hh + 64] = W_qkv[:, cols]
            bvv[hh] = b_qkv[cols]
        wv = wv.reshape(4, 2, 128, 256).transpose(2, 0, 1, 3)
        wv8 = wv.astype(E4)
        wv8l = ((wv - wv8.astype(np.float32)) * 8.0).astype(E4)
        wv8s = (wv * 0.125).astype(E4)
        bv2 = np.broadcast_to(bvv[None], (128, 4, 64))

        wo = W_out[g * 256:(g + 1) * 256, :]             # [256, 1024]
        wob = wo.reshape(2, 128, D).transpose(1, 0, 2).astype(BF)

        in_maps.append({
            "xq8": x8,
            "xq8l": x8l,
            "xs8": xs8_a,
            "zro": np.zeros((128, 384), np.float32),
            "wq8": np.ascontiguousarray(wq8),
            "wq8l": np.ascontiguousarray(wq8l),
            "wq8s": np.ascontiguousarray(wq8s),
            "wv8": np.ascontiguousarray(wv8),
            "wv8l": np.ascontiguousarray(wv8l),
            "wv8s": np.ascontiguousarray(wv8s),
            "woutb": np.ascontiguousarray(wob),
            "bq": np.ascontiguousarray(bqv),
            "bv": np.ascontiguousarray(bv2),
            "vone": vone,
            "cmask": np.ascontiguousarray(cmask),
            "identb": identb,
        })

    _CACHE["in_maps"] = in_maps
    res = bass_utils.run_bass_kernel_spmd(nc, in_maps, core_ids=list(range(N_CORES)))

    out = np.zeros((B, S, D), np.float32)
    for c in range(N_CORES):
        b = c // 4
        oT = np.asarray(res.results[c]["outT"]).astype(np.float32)
        out[b] += oT.transpose(1, 0, 2).reshape(D, S).T
    out += b_out
    return out


# revision 5
# speedup vs baseline: 1.0300x; 1.0019x over previous
"""Multi-head causal attention (B=2, S=2048, D=1024, H=16) on 8 TRN2 NeuronCores.
135.3us TimelineSim (baseline 184.4us).

Sharding: core c handles batch b = c // 4 and local head group g = c % 4
(global heads 4g..4g+3).  Each core computes its heads' QKV projections,
causal attention, and a partial output projection; host sums the 4 partials
per batch and adds b_out.

v2 design (vs baseline):
  - QK projection: fp8e4 DoubleRow matmuls, 3-pass hi/lo error compensation
    (x8@W8 + x8lo@W8 + x8@W8lo) -> q,k accurate to ~0.4%; f32r scores.
  - V projection: same 3-pass fp8 DR -> vn in bf16 (k-major, +ones col).
  - Scores: f32r, two heads packed per 128-partition psum tile, causal
    trimming at 256 granularity (f32r needs moving dim >= 256).
  - exp on ACT -> bf16 E tiles; triangle masks on DVE (bf16 2x mode).
  - AV transposed: out[q(128), 65] = E_block[k,q].T @ Vn[k, 65]; moving dim
    is only 65 cols -> ~2x fewer PE cycles than value-major AV.  Ones column
    of Vn gives the softmax denominator per q ON THE PARTITION, so
    normalization is a per-partition reciprocal + tensor_scalar multiply
    (no cross-partition broadcast needed at all).
  - values transposed back to [d, q] with PE transpose matmuls (bf16),
    bf16 output projection.
"""

from contextlib import ExitStack

import numpy as np
import ml_dtypes

import concourse.bass as bass
import concourse.mybir as mybir
import concourse.tile as tile
from concourse import bass_utils

F32 = mybir.dt.float32
F32R = mybir.dt.float32r
BF16 = mybir.dt.bfloat16
FP8 = mybir.dt.float8e4
EXP = mybir.ActivationFunctionType.Exp
COPY = mybir.ActivationFunctionType.Copy
DR = mybir.MatmulPerfMode.DoubleRow

E4 = ml_dtypes.float8_e4m3
BF = ml_dtypes.bfloat16

B, S, D, H = 2, 2048, 1024, 16
HD = D // H          # 64
HL = 4               # heads per core
N_CORES = 8
SC = S // 512        # 4 q-chunks of 512
KT = S // 128        # 16 k-tiles of 128

_CACHE = {}


def _round_f32r(x: np.ndarray) -> np.ndarray:
    """Round f32 to fp32r (11-bit mantissa, RNE) on host."""
    u = np.ascontiguousarray(x, dtype=np.float32).view(np.uint32)
    frac = u & np.uint32(0x00000FFF)
    base = u & np.uint32(0xFFFFF000)
    bit = np.uint32(0x00000800)
    lsb = np.uint32(0x00001000)
    roundup = (frac > bit) | ((frac == bit) & ((u & lsb) != 0))
    return np.where(roundup, base + lsb, base).view(np.float32)


_NO_HOIST = {
    "AllEngineBarrier",
    "EventSemaphore",
    "UnconditionalBranch",
    "CompareAndBranch",
    "BranchHint",
    "IndirectBranch",
    "Halt",
    "Call",
    "OverlayCall",
    "NoOp",
}


def _fix_sync_waits(nc):
    """walrus codegen holds only one sync-wait per engine instruction; hoist
    excess waits onto same-engine NoOps inserted right before."""
    for fn in nc.m.functions:
        for blk in fn.blocks:
            insts = blk.instructions
            out = []
            changed = False
            for inst in insts:
                si = inst.sync_info
                if si is not None and inst.opcode not in _NO_HOIST:
                    waits = list(si.on_wait)
                    if len(waits) > 1:
                        for j, w in enumerate(waits[:-1]):
                            nop = mybir.InstNoOp(name=f"{inst.name}-wfix{j}")
                            nop.engine = inst.engine
                            nop.sync_info = mybir.SyncInfo(on_wait=[w], on_update=[])
                            out.append(nop)
                        inst.sync_info = mybir.SyncInfo(
                            on_wait=[waits[-1]], on_update=list(si.on_update)
                        )
                        changed = True
                out.append(inst)
            if changed:
                blk.instructions = out


def _build(fix_waits=True, dbg=False):
    nc = bass.Bass("TRN2", target_bir_lowering=False, debug=False,
                   num_devices=N_CORES)
    if dbg:
        d_qT = nc.dram_tensor("d_qT", [128, 2, S], F32R, kind="ExternalOutput").ap()
        d_kT = nc.dram_tensor("d_kT", [128, 2, S], F32R, kind="ExternalOutput").ap()
        d_vn = nc.dram_tensor("d_vn", [128, KT, 4, 65], BF16,
                              kind="ExternalOutput").ap()
        d_e = nc.dram_tensor("d_e", [128, 2, 512], BF16, kind="ExternalOutput").ap()
        d_vst = nc.dram_tensor("d_vst", [128, 4, 4, 64], BF16,
                               kind="ExternalOutput").ap()
        d_vnT = nc.dram_tensor("d_vnT", [128, 2, S], BF16,
                               kind="ExternalOutput").ap()

    xq8 = nc.dram_tensor("xq8", [128, 4, 2, S], FP8, kind="ExternalInput").ap()
    xq8l = nc.dram_tensor("xq8l", [128, 4, 2, S], FP8, kind="ExternalInput").ap()
    xs8 = nc.dram_tensor("xs8", [128, 4, 2, S], FP8, kind="ExternalInput").ap()
    zro = nc.dram_tensor("zro", [128, 384], F32R, kind="ExternalInput").ap()
    wq8 = nc.dram_tensor("wq8", [128, 4, 2, 4, 128], FP8, kind="ExternalInput").ap()
    wq8l = nc.dram_tensor("wq8l", [128, 4, 2, 4, 128], FP8, kind="ExternalInput").ap()
    wq8s = nc.dram_tensor("wq8s", [128, 4, 2, 4, 128], FP8, kind="ExternalInput").ap()
    wv8 = nc.dram_tensor("wv8", [128, 4, 2, 256], FP8, kind="ExternalInput").ap()
    wv8l = nc.dram_tensor("wv8l", [128, 4, 2, 256], FP8, kind="ExternalInput").ap()
    wv8s = nc.dram_tensor("wv8s", [128, 4, 2, 256], FP8, kind="ExternalInput").ap()
    woutb = nc.dram_tensor("woutb", [128, 2, D], BF16, kind="ExternalInput").ap()
    bq = nc.dram_tensor("bq", [128, 4], F32, kind="ExternalInput").ap()
    bv = nc.dram_tensor("bv", [128, 4, 64], F32, kind="ExternalInput").ap()
    vone = nc.dram_tensor("vone", [128, KT, 4, 1], BF16, kind="ExternalInput").ap()
    cmask = nc.dram_tensor("cmask", [128, 128], BF16, kind="ExternalInput").ap()
    identb = nc.dram_tensor("identb", [128, 128], BF16, kind="ExternalInput").ap()
    outT = nc.dram_tensor("outT", [128, 8, S], BF16, kind="ExternalOutput").ap()

    with tile.TileContext(nc) as tc, ExitStack() as ctx:
        persist = ctx.enter_context(tc.tile_pool(name="persist", bufs=1))
        xpool = ctx.enter_context(tc.tile_pool(name="xp", bufs=3))
        epool = ctx.enter_context(tc.tile_pool(name="ep", bufs=8))
        spool = ctx.enter_context(tc.tile_pool(name="stp", bufs=3))
        opool = ctx.enter_context(tc.tile_pool(name="op", bufs=6))
        # psum (8 banks): sp 2x2-bank, po 1x2-bank, small (pq/pv/pu/tr) 2x1
        ps = ctx.enter_context(tc.tile_pool(name="ps", bufs=2, space="PSUM"))

        wq_sb = persist.tile([128, 4, 2, 4, 128], FP8, tag="wq")
        wql_sb = persist.tile([128, 4, 2, 4, 128], FP8, tag="wql")
        wqs_sb = persist.tile([128, 4, 2, 4, 128], FP8, tag="wqs")
        wv_sb = persist.tile([128, 4, 2, 256], FP8, tag="wv")
        wvl_sb = persist.tile([128, 4, 2, 256], FP8, tag="wvl")
        wvs_sb = persist.tile([128, 4, 2, 256], FP8, tag="wvs")
        zro_sb = persist.tile([128, 384], F32R, tag="zro")
        wo_sb = persist.tile([128, 2, D], BF16, tag="wo")
        bq_sb = persist.tile([128, 4], F32, tag="bq")
        bv_sb = persist.tile([128, 4, 64], F32, tag="bv")
        cm_sb = persist.tile([128, 128], BF16, tag="cm")
        id_sb = persist.tile([128, 128], BF16, tag="id")
        qT = persist.tile([128, 2, S], F32R, tag="qT")
        kT = persist.tile([128, 2, S], F32R, tag="kT")
        vn = persist.tile([128, KT, 4, 65], BF16, tag="vn")
        vnT = persist.tile([128, 2, S], BF16, tag="vnT")

        # first x chunk first so the first matmuls start early
        xc0 = xpool.tile([128, 4, 2, 512], FP8, tag="xc", name="xc0")
        xl0 = xpool.tile([128, 4, 2, 512], FP8, tag="xl", name="xl0")
        xs0 = xpool.tile([128, 4, 2, 512], FP8, tag="xs", name="xs0")
        nc.sync.dma_start(xc0[:], xq8[:, :, :, 0:512])
        nc.scalar.dma_start(bq_sb[:], bq)
        nc.scalar.dma_start(wq_sb[:], wq8)
        nc.sync.dma_start(xs0[:], xs8[:, :, :, 0:512])
        nc.scalar.dma_start(wql_sb[:], wq8l)
        nc.sync.dma_start(xl0[:], xq8l[:, :, :, 0:512])
        nc.scalar.dma_start(wqs_sb[:], wq8s)
        nc.scalar.dma_start(bv_sb[:], bv)
        nc.scalar.dma_start(wv_sb[:], wv8)
        nc.scalar.dma_start(wvl_sb[:], wv8l)
        nc.scalar.dma_start(wvs_sb[:], wv8s)
        nc.scalar.dma_start(cm_sb[:], cmask)
        nc.scalar.dma_start(zro_sb[:], zro)
        nc.scalar.dma_start(id_sb[:], identb)
        nc.scalar.dma_start(wo_sb[:], woutb)
        # ones column of vn (softmax denominators) via memset, not DMA
        # (a strided single-element-column DMA costs ~3.6us of DMA engines)
        nc.vector.memset(vn[:, :, :, 64:65], 1.0)
        xtiles = {0: (xc0, xl0, xs0)}

        def qkv_dma(qc):
            qs = slice(qc * 512, (qc + 1) * 512)
            xc = xpool.tile([128, 4, 2, 512], FP8, tag="xc", name=f"xc{qc}")
            xl = xpool.tile([128, 4, 2, 512], FP8, tag="xl", name=f"xl{qc}")
            xs = xpool.tile([128, 4, 2, 512], FP8, tag="xs", name=f"xs{qc}")
            nc.sync.dma_start(xc[:], xq8[:, :, :, qs])
            nc.sync.dma_start(xl[:], xq8l[:, :, :, qs])
            nc.sync.dma_start(xs[:], xs8[:, :, :, qs])
            xtiles[qc] = (xc, xl, xs)

        def qk_tile(qc, mt):
            qs = slice(qc * 512, (qc + 1) * 512)
            xc, xl, xs = xtiles[qc]
            pq = ps.tile([128, 512], F32, tag="q1", name=f"pq{qc}{mt}")
            passes = [(wq_sb, xc), (wql_sb, xs), (wqs_sb, xl)]
            i = 0
            for wsb, xsb in passes:
                for kp in range(4):
                    nc.tensor.matmul(
                        pq[:], wsb[:, kp, :, mt, :], xsb[:, kp, :, :],
                        start=(i == 0), stop=(i == 11), perf_mode=DR)
                    i += 1
            dst = (qT if mt < 2 else kT)[:, mt % 2, qs]
            nc.vector.tensor_scalar_add(dst, pq[:], bq_sb[:, mt:mt + 1])

        def v_tile(qc, j):
            st = 4 * qc + j
            xc, xl, xs = xtiles[qc]
            pv = ps.tile([128, 512], F32, tag="q1", name=f"pv{qc}{j}")
            passes = [(wv_sb, xc), (wvl_sb, xs), (wvs_sb, xl)]
            i = 0
            for wsb, xsb in passes:
                for kp in range(4):
                    nc.tensor.matmul(
                        pv[0:128, 0:256], xsb[:, kp, :, j * 128:(j + 1) * 128],
                        wsb[:, kp, :, :],
                        start=(i == 0), stop=(i == 11), perf_mode=DR)
                    i += 1
            nc.vector.tensor_add(
                vn[:, st, :, 0:64],
                pv[0:128, 0:256].rearrange("p (h d) -> p h d", h=4),
                bv_sb[:])

        def qkv_pieces(qc):
            ps_ = [lambda qc=qc: qkv_dma(qc)] if qc > 0 else []
            for mt in (0, 2):
                ps_.append(lambda qc=qc, mt=mt: qk_tile(qc, mt))
            for j in range(4):
                ps_.append(lambda qc=qc, j=j: v_tile(qc, j))
            for mt in (1, 3):
                ps_.append(lambda qc=qc, mt=mt: qk_tile(qc, mt))
            return ps_

        vst_tiles = {}

        def tr_piece(qc, qt):
            vst = vst_tiles[qc]
            for dh in range(2):
                ptr = ps.tile([128, 128], BF16, tag="q1", name=f"tr{qc}{qt}{dh}")
                nc.tensor.matmul(ptr[:], vst[:, qt, 2 * dh:2 * dh + 2, :],
                                 id_sb[:], is_transpose=True)
                nc.vector.tensor_copy(
                    vnT[:, dh, qc * 512 + qt * 128:qc * 512 + (qt + 1) * 128],
                    ptr[:])

        def op_piece(qc, m):
            qs = slice(qc * 512, (qc + 1) * 512)
            pu = ps.tile([128, 512], F32, tag="q1", name=f"pu{qc}{m}")
            for t in range(2):
                nc.tensor.matmul(pu[:], wo_sb[:, t, m * 128:(m + 1) * 128],
                                 vnT[:, t, qs], start=(t == 0), stop=(t == 1))
            ou = opool.tile([128, 512], BF16, tag="ou", name=f"ou{qc}{m}")
            if qc == SC - 1 and m % 2 == 1:
                nc.scalar.copy(ou[:], pu[:])   # tail: ACT is idle
            else:
                nc.vector.tensor_copy(ou[:], pu[:])
            nc.sync.dma_start(outT[:, m, qs], ou[:])

        def post_pieces(qc):
            ps_ = [lambda qc=qc, qt=qt: tr_piece(qc, qt) for qt in range(4)]
            ps_ += [lambda qc=qc, m=m: op_piece(qc, m) for m in range(8)]
            return ps_

        def attn_qc(qc, queue):
            """Emit attention for chunk qc, interleaving `queue` pieces (PE
            work for the next chunk's projections and the previous chunk's
            transposes/output projection) between ki steps so every engine
            stays fed while the exp (ACT) chain runs."""
            vst = spool.tile([128, 4, 4, 64], BF16, tag="vst", name=f"vs{qc}")
            vst_tiles[qc] = vst
            n_ki = 4 * qc + 4
            n_steps = 2 * n_ki + 2
            qi = 0
            emitted = 0.0

            def drain(frac):
                nonlocal qi, emitted
                emitted += frac
                while qi < len(queue) and qi < emitted:
                    queue[qi]()
                    qi += 1

            per_step = 0.5 * len(queue) / n_steps
            for hp in range(2):
                po = ps.tile([128, 2, 512], F32, tag="po", name=f"po{qc}{hp}", bufs=1)
                for i in range(2):
                    # one start=True matmul zeroes all four qt accumulation
                    # regions of this bank (psum pending-zero is bank-wide)
                    nc.tensor.matmul(po[:, i, 0:260], zro_sb[0:1, 0:128],
                                     zro_sb[0:1, 0:260], start=True, stop=False,
                                     skip_group_check=True)
                for ki in range(n_ki):
                    j = ki - 4 * qc  # >= 0 on diagonal tiles
                    o_exp = max(0, 128 * j)
                    o_sc = min(o_exp, 256)  # f32r moving dim must be >= 256
                    sp = ps.tile([128, 2, 512], F32, tag="s",
                                 name=f"sp{qc}{hp}{ki}")
                    for i in range(2):
                        vp = 64 * i
                        nc.tensor.matmul(
                            sp[:, i, o_sc:512],
                            kT[vp:vp + 64, hp, ki * 128:(ki + 1) * 128],
                            qT[vp:vp + 64, hp, qc * 512 + o_sc:(qc + 1) * 512],
                            start=True, stop=True, tile_position=(vp, 0))
                    e = epool.tile([128, 2, 512], BF16, tag="e",
                                   name=f"e{qc}{hp}{ki}")
                    nc.scalar.activation(e[:, :, o_exp:512], sp[:, :, o_exp:512],
                                         EXP, scale=0.125)
                    if j >= 0:  # diagonal: mask the [128,128] triangle block
                        for i in range(2):
                            es = e[:, i, o_exp:o_exp + 128]
                            nc.vector.tensor_mul(es, es, cm_sb[:])
                    # AV transposed: po[q, 65] += E_block.T @ Vn
                    for i in range(2):
                        for qt in range(max(0, j), 4):
                            nc.tensor.matmul(
                                po[:, i, qt * 65:qt * 65 + 65],
                                e[:, i, qt * 128:(qt + 1) * 128],
                                vn[:, ki, 2 * hp + i, :],
                                start=False, stop=(ki == 4 * qc + qt),
                                skip_group_check=True)
                    drain(per_step)
                # normalize: per-partition recip of denominators, then scale
                rc = spool.tile([128, 2, 4], F32, tag="rc", name=f"rc{qc}{hp}")
                for i in range(2):
                    dn = po[:, i, 0:260].rearrange("p (qt c) -> p qt c", c=65)
                    with nc.allow_low_precision(reason="softmax recip"):
                        nc.vector.reciprocal(rc[:, i, :], dn[:, 0:4, 64:65])
                    for qt in range(4):
                        nc.vector.tensor_scalar_mul(
                            vst[:, qt, 2 * hp + i, :],
                            po[:, i, qt * 65:qt * 65 + 64],
                            rc[:, i, qt:qt + 1])
                drain(0.0)
            drain(len(queue))

        qk_tile(0, 0)
        qk_tile(0, 2)
        for j in range(4):
            v_tile(0, j)
        for c in range(SC):
            queue = []
            if c == 0:
                queue += [lambda mt=mt: qk_tile(0, mt) for mt in (1, 3)]
            if c >= 1:
                queue += post_pieces(c - 1)
            if c + 1 < SC:
                queue += qkv_pieces(c + 1)
            attn_qc(c, queue)
        for piece in post_pieces(SC - 1):
            piece()
        if dbg:
            nc.sync.dma_start(d_vst, vst_tiles[0][:])
            nc.sync.dma_start(d_qT, qT[:])
            nc.sync.dma_start(d_kT, kT[:])
            nc.sync.dma_start(d_vn, vn[:])
            nc.sync.dma_start(d_vnT, vnT[:])

    if fix_waits:
        _fix_sync_waits(nc)
    return nc


def _get_nc():
    if "nc" not in _CACHE:
        _CACHE["nc"] = _build()
    return _CACHE["nc"]


def _dr_layout(xb):
    """[S, 1024] -> [128, 4, 2, S]: p=partition, kp=k-tile-pair, sl=slot."""
    return np.ascontiguousarray(
        xb.T.reshape(4, 2, 128, xb.shape[0]).transpose(2, 0, 1, 3))


def kernel(x, W_qkv, b_qkv, W_out, b_out):
    x = np.asarray(x, np.float32)
    W_qkv = np.asarray(W_qkv, np.float32)
    b_qkv = np.asarray(b_qkv, np.float32)
    W_out = np.asarray(W_out, np.float32)
    b_out = np.asarray(b_out, np.float32)

    nc = _get_nc()

    kk = np.arange(128)[:, None]
    qq = np.arange(128)[None, :]
    cmask = (kk <= qq).astype(BF)
    identb = np.eye(128, dtype=np.float32).astype(BF)
    vone = np.ones((128, KT, 4, 1), np.float32).astype(BF)

    in_maps = []
    for c in range(N_CORES):
        b, g = divmod(c, 4)
        heads = [4 * g + i for i in range(HL)]

        xb = x[b]                                        # [S, 1024]
        xr = _dr_layout(xb)
        x8 = xr.astype(E4)
        x8l = ((xr - x8.astype(np.float32)) * 8.0).astype(E4)
        xs8_a = (xr * 0.125).astype(E4)

        # qk weight m-tiles: mt0=q-hp0, mt1=q-hp1, mt2=k-hp0, mt3=k-hp1
        # out-col within tile = 64*i + dd  (i head-in-pair, dd hd index)
        wq = np.zeros((1024, 4, 128), np.float32)
        bqv = np.zeros((128, 4), np.float32)
        for mt in range(4):
            t, hp = divmod(mt, 2)       # t: 0=q, 1=k
            for i in range(2):
                h = heads[2 * hp + i]
                cols = h * 192 + 64 * t + np.arange(64)
                wq[:, mt, 64 * i:64 * i + 64] = W_qkv[:, cols]
                bqv[64 * i:64 * i + 64, mt] = b_qkv[cols]
        wq = wq.reshape(4, 2, 128, 4, 128).transpose(2, 0, 1, 3, 4)
        wq8 = wq.astype(E4)
        wq8l = ((wq - wq8.astype(np.float32)) * 8.0).astype(E4)
        wq8s = (wq * 0.125).astype(E4)

        # v weights: col = 64*h + dd
        wv = np.zeros((1024, 256), np.float32)
        bvv = np.zeros((4, 64), np.float32)
        for hh in range(4):
            cols = heads[hh] * 192 + 128 + np.arange(64)
            wv[:, 64 * hh:64 * hh + 64] = W_qkv[:, cols]
            bvv[hh] = b_qkv[cols]
        wv = wv.reshape(4, 2, 128, 256).transpose(2, 0, 1, 3)
        wv8 = wv.astype(E4)
        wv8l = ((wv - wv8.astype(np.float32)) * 8.0).astype(E4)
        wv8s = (wv * 0.125).astype(E4)
        bv2 = np.broadcast_to(bvv[None], (128, 4, 64))

        wo = W_out[g * 256:(g + 1) * 256, :]             # [256, 1024]
        wob = wo.reshape(2, 128, D).transpose(1, 0, 2).astype(BF)

        in_maps.append({
            "xq8": x8,
            "xq8l": x8l,
            "xs8": xs8_a,
            "zro": np.zeros((128, 384), np.float32),
            "wq8": np.ascontiguousarray(wq8),
            "wq8l": np.ascontiguousarray(wq8l),
            "wq8s": np.ascontiguousarray(wq8s),
            "wv8": np.ascontiguousarray(wv8),
            "wv8l": np.ascontiguousarray(wv8l),
            "wv8s": np.ascontiguousarray(wv8s),
            "woutb": np.ascontiguousarray(wob),
            "bq": np.ascontiguousarray(bqv),
            "bv": np.ascontiguousarray(bv2),
            "vone": vone,
            "cmask": np.ascontiguousarray(cmask),
            "identb": identb,
        })

    _CACHE["in_maps"] = in_maps
    res = bass_utils.run_bass_kernel_spmd(nc, in_maps, core_ids=list(range(N_CORES)))

    out = np.zeros((B, S, D), np.float32)
    for c in range(N_CORES):
        b = c // 4
        oT = np.asarray(res.results[c]["outT"]).astype(np.float32)
        out[b] += oT.transpose(1, 0, 2).reshape(D, S).T
    out += b_out
    return out
